# revision 24
# baseline (speedup 1.0000x reference)
"""Depth-masked 3-branch 3x3 conv (Conv2.5D) on 8 TRN2 NeuronCores.

Data-parallel over N=8 images (1 image/core). Per core:
  - phi in {0,1,2,3} encodes which branch is active per (tap,pixel)
    (phi = r for r in {1,2,3}, else 0), computed compactly
  - per-pattern branch tiles use mixed polynomial bases over phi:
      p0,p1: {phi*x, E2*x, E3*x}           E_j = (phi >= j-.5), 0/1
      p2:    {phi*x, E2*x, s3*x, x}        s_j = sign(phi-(j-.5)), +-1
      p3:    {phi*x, s2*x, s3*x, x}
    with host-transformed weights; raw-x groups are free (window rhs),
    sign planes are built by the Activation engine (AF.Sign) straight
    from the PE-broadcast PSUM
  - broadcasts: p0 phi via DVE stream_shuffle (pipelined one chunk
    ahead so Pool's tensor_tensors start at cycle zero); p1,p2,p3 phi
    via PE selection-matmul + Act copy
  - tile multiplies: 9 on DVE (2x tensor_tensor), 3 on Pool
  - tap pairs (0,1),(7,8) share tile tA=[x; x+1]; (2,5),(3,6) share
    tB=[x; x+128]; center tap 4 is always branch-1 -> free GEMM group
  - 30 K=128 bf16 matmuls per 1024-px chunk
"""

import sys

sys.path.insert(0, "/opt/trn_rl_repo")

import numpy as np
import ml_dtypes

import concourse.bass as bass
import concourse.mybir as mybir
from concourse.bass_utils import run_bass_kernel_spmd
from concourse import tile
from concourse.vector_clock import VectorClock, ScopedClock

F32 = mybir.dt.float32
BF16 = mybir.dt.bfloat16
AF = mybir.ActivationFunctionType
ALU = mybir.AluOpType

N_IMG, C, O, H, W = 8, 64, 64, 128, 128
L = H * W
CHUNK = 1024
NCHUNK = L // CHUNK
BASE = 256  # pad on each side of the x tiles (reads span +-129)
# tap k = 3*(dh+1)+(dw+1); flat pixel offset dh*W+dw
OFF = [(k // 3 - 1) * W + (k % 3 - 1) for k in range(9)]
# tap pairs (ka,kb): off(kb)-off(ka) == 1 -> tA, == 128 -> tB
PAIRS = [(0, 1), (7, 8), (2, 5), (3, 6)]
PAIR_SRC = ["A", "A", "B", "B"]
NGRP = 15  # center + p0(3) + p1(3) + p2(4) + p3(4)
# rc rows 0-5 = phi taps (7,8,2,5,3,6) for PE broadcasts of p1,p2,p3;
# p0 shuffle-source rows at mask index 8 per bank
SHUF_ROWS = [8, 40, 72, 104]
SHUF_MASK = 8


def _patched_drain_and_barrier(self, tick_clock, wait_clock):
    # stock version puts every live sem wait on one drain -> walrus
    # "Too many sync wait commands"; emit one single-wait NOP per sem.
    ticks = list(tick_clock.global_clock)
    n = len(ticks)
    for i, t in enumerate(ticks):
        if t > 0:
            vec = [0] * n
            vec[i] = t
            nop = self.nc.sync.nop()
            wait_clock.add_sem_waits(nop.ins, ScopedClock({None: VectorClock(vec)}))
    self.nc.sync.drain()
    self.nc.all_engine_barrier()
    popped = self.nc._tile_sem_poison_stack.pop()
    assert popped is self._sem_poison
    self.nc.clear_and_free_semaphores(list(self.sems.allocated().values()))
    self.nc.all_engine_barrier()


tile.TileContext._drain_and_barrier = _patched_drain_and_barrier


def _split_excess_waits(nc, noop_cls, max_waits=1):
    # this walrus build rejects >1 sync-wait on several instruction
    # structs; hoist extras onto same-engine NoOps placed just before.
    for fn in nc.m.functions:
        for blk in fn.blocks:
            idx = 0
            while idx < len(blk.instructions):
                inst = blk.instructions[idx]
                si = inst.sync_info
                if si is not None and len(si.on_wait) > max_waits:
                    waits = list(si.on_wait)
                    si.on_wait = waits[-max_waits:]
                    pos = idx
                    for w in waits[:-max_waits]:
                        nop = noop_cls(
                            name=nc.get_next_instruction_name(), ins=[], outs=[]
                        )
                        nop.engine = inst.engine
                        nop.sync_info = mybir.SyncInfo(on_wait=[w], on_update=[])
                        nc.register_instruction(nop)
                        blk.instructions.insert(pos, nop)
                        pos += 1
                        idx += 1
                idx += 1


def _build_graph():
    nc = bass.Bass()
    x_d = nc.declare_dram_parameter("x", [C, L], F32, isOutput=False)
    dep_d = nc.declare_dram_parameter("depth", [H, W], F32, isOutput=False)
    rfx_d = nc.declare_dram_parameter("rfx", [128, 1], F32, isOutput=False)
    wp_d = nc.declare_dram_parameter("wp", [128, NGRP * 64], BF16, isOutput=False)
    sel_d = nc.declare_dram_parameter("sel", [6, 384], BF16, isOutput=False)
    out_d = nc.declare_dram_parameter("out", [O, L], F32, isOutput=True)

    XW = BASE + L + BASE
    with tile.TileContext(nc) as tc:
        with (
            tc.tile_pool(name="big", bufs=1) as big,
            tc.tile_pool(name="mask", bufs=2) as mk,
            tc.tile_pool(name="stage", bufs=2) as stage,
            tc.tile_pool(name="rrep", bufs=7) as rrp,
            tc.tile_pool(name="xm", bufs=6) as xmp,
            tc.tile_pool(name="eb", bufs=5) as ebp,
            tc.tile_pool(name="outp", bufs=2) as outp,
            tc.tile_pool(name="psum", bufs=2, space=bass.MemorySpace.PSUM) as psp,
            tc.tile_pool(name="psb", bufs=2, space=bass.MemorySpace.PSUM) as psb,
        ):
            wp = big.tile([128, NGRP * 64], BF16)
            nc.sync.dma_start(wp[:], wp_d[:])
            sel = big.tile([6, 384], BF16)
            nc.sync.dma_start(sel[:], sel_d[:])

            dsh = mk.tile([128, 3 * 130], F32)
            nc.vector.memset(dsh[:], 0.0)
            nc.sync.dma_start(dsh[:, 131:259], dep_d[:, :])
            nc.sync.dma_start(dsh[0:127, 261:389], dep_d[1:128, :])
            nc.sync.dma_start(dsh[1:128, 1:129], dep_d[0:127, :])
            rfx = mk.tile([128, 1], F32)
            nc.sync.dma_start(rfx[:], rfx_d[:])
            # per-partition bias vectors for AF.Sign thresholds
            b15 = mk.tile([128, 1], F32)
            nc.vector.memset(b15[:], -1.5)
            b25 = mk.tile([128, 1], F32)
            nc.vector.memset(b25[:], -2.5)

            # ---- x tiles staged early (HWDGE not serialized behind the
            # mask-row collapse DMAs); bf16 conversion on Pool ----
            tA = big.tile([128, XW], BF16)
            tB = big.tile([128, XW], BF16)
            nc.vector.memset(tA[:, 0:BASE], 0.0)
            nc.vector.memset(tA[:, BASE + L - 132 : XW], 0.0)
            nc.gpsimd.memset(tB[:, 0:BASE], 0.0)
            nc.gpsimd.memset(tB[:, BASE + L - 132 : XW], 0.0)

            def stage_chunk(ci):
                c0 = ci * CHUNK
                xs = stage.tile([C, CHUNK], F32, tag="xs")
                nc.sync.dma_start(xs[:], x_d[:, c0 : c0 + CHUNK])
                up = tA[0:64, BASE + c0 : BASE + c0 + CHUNK]
                nc.gpsimd.tensor_copy(up, xs[:])
                nc.sync.dma_start(
                    tA[64:128, BASE + c0 - 1 : BASE + c0 - 1 + CHUNK], up
                )
                nc.sync.dma_start(
                    tB[64:128, BASE + c0 - 128 : BASE + c0 - 128 + CHUNK], up
                )
                nc.sync.dma_start(tB[0:64, BASE + c0 : BASE + c0 + CHUNK], up)

            stage_chunk(0)
            stage_chunk(1)

            # ---- depth -> phi encoding (128h x 9*128w), phi in {0..3} ----
            g = mk.tile([128, 128], F32)
            nc.vector.tensor_scalar(g[:], dsh[:, 131:259], rfx[:], None, ALU.mult)
            rg = mk.tile([128, 128], F32)
            nc.vector.reciprocal(rg[:], g[:])

            def _win(base, offset, dims):
                return bass.AP(
                    base.tensor, offset, [list(base.ap[0])] + [list(d) for d in dims]
                )

            dcol = _win(dsh[:], 0, [(130, 3), (1, 3), (1, 128)])
            cent = _win(dsh[:], 131, [(0, 3), (0, 3), (1, 128)])
            rgb = _win(rg[:], 0, [(0, 9), (1, 128)])

            et = mk.tile([128, 9 * 128], F32)
            nc.vector.tensor_tensor(et[:], dcol, cent, ALU.subtract)
            tt = mk.tile([128, 9 * 128], F32)
            nc.vector.tensor_tensor(tt[:], et[:], rgb, ALU.mult)
            ua = mk.tile([128, 9 * 128], F32, tag="u")
            nc.vector.tensor_scalar(ua[:], tt[:], -1.5, None, ALU.is_ge)
            ub = mk.tile([128, 9 * 128], F32, tag="u")
            nc.vector.scalar_tensor_tensor(ub[:], tt[:], -0.5, ua[:], ALU.is_ge, ALU.add)
            uc = mk.tile([128, 9 * 128], F32, tag="u")
            nc.vector.scalar_tensor_tensor(uc[:], tt[:], 0.5, ub[:], ALU.is_ge, ALU.add)
            nc.vector.tensor_scalar(et[:], tt[:], 1.5, -3.0, ALU.is_ge, ALU.mult)
            renc = mk.tile([128, 9 * 128], BF16)
            nc.vector.tensor_tensor(renc[:], uc[:], et[:], ALU.add)

            # ---- rc: PE rows 0-5 first (first chunk needs them), then
            # p0 shuffle-source rows; split across HWDGE/SWDGE queues ----
            rc = big.tile([128, L], BF16)
            coll = [(0, 7), (1, 8), (2, 2), (3, 5), (4, 3), (5, 6)]
            coll += [(r, PAIRS[0][0 if i < 2 else 1]) for i, r in enumerate(SHUF_ROWS)]
            for i, (r, k) in enumerate(coll):
                eng = nc.sync if i % 2 == 0 else nc.gpsimd
                eng.dma_start(rc[r : r + 1, :], renc[:, k * 128 : (k + 1) * 128])

            # p0 shuffle + indicator pipeline runs ONE CHUNK AHEAD so
            # Pool's tensor_tensors have their inputs at cycle start.
            def emit_shuf_ts(ci):
                c0 = ci * CHUNK
                rr = rrp.tile([128, CHUNK], BF16, tag="rr")
                nc.vector.stream_shuffle(
                    rr[:], rc[:, c0 : c0 + CHUNK], mask=[SHUF_MASK] * 32
                )
                ebs = {}
                for j in (2, 3):
                    eb = ebp.tile([128, CHUNK], BF16, tag="eb")
                    nc.vector.tensor_scalar(eb[:], rr[:], j - 0.5, None, ALU.is_ge)
                    ebs[j] = eb
                return rr, ebs

            pipe = emit_shuf_ts(0)

            # logical groups: 0=center, 1-3 p0{f1,f2,f3}, 4-6 p1{f1,f2,f3},
            # 7-10 p2{f1,f2,f3,xr}, 11-14 p3{f1,f2,f3,xr};
            # emission order ~ tile readiness ("xr"/"c" are free windows)
            MM_ORDER = [0, 10, 14, 1, 4, 2, 6, 7, 3, 8, 5, 11, 9, 12, 13]

            for ci in range(NCHUNK):
                c0 = ci * CHUNK
                shuf_rr, pool_eb = pipe

                def xwin_of(p):
                    src = tA if PAIR_SRC[p] == "A" else tB
                    woff = OFF[PAIRS[p][0]]
                    return src[:, BASE + c0 + woff : BASE + c0 + woff + CHUNK]

                xms = {}

                # ---- Pool tiles first: inputs were built last chunk ----
                for j in (2, 3):  # p0 f2 (gid 2), f3 (gid 3)
                    xm = xmp.tile([128, CHUNK], BF16, tag="xm")
                    nc.gpsimd.tensor_tensor(
                        xm[:], pool_eb[j][:], xwin_of(0), ALU.mult
                    )
                    xms[j] = xm

                # ---- PE phi broadcasts for p1,p2,p3 + Act copies/signs ----
                rrpe = {}
                rps_of = {}
                for bi, p in enumerate((1, 2, 3)):
                    rps = psb.tile([128, CHUNK], F32, tag="rps")
                    for h in range(2):
                        nc.tensor.matmul(
                            rps[:, h * 512 : (h + 1) * 512],
                            sel[:, bi * 128 : (bi + 1) * 128],
                            rc[0:6, c0 + h * 512 : c0 + (h + 1) * 512],
                            start=True, stop=True,
                        )
                    rr = rrp.tile([128, CHUNK], BF16, tag="rr")
                    nc.scalar.activation(rr[:], rps[:], AF.Copy)
                    rrpe[p] = rr
                    rps_of[p] = rps

                # sign planes from PSUM: s_j = sign(phi - (j-0.5))
                sg = {}
                for p, j in ((2, 3), (3, 2), (3, 3)):
                    s = rrp.tile([128, CHUNK], BF16, tag="rr")
                    nc.scalar.activation(
                        s[:], rps_of[p][:], AF.Sign,
                        bias=(b25 if j == 3 else b15)[:],
                    )
                    sg[(p, j)] = s

                if ci + 2 < NCHUNK:
                    stage_chunk(ci + 2)

                # ---- DVE tiles ----
                def dve_tt(gid, a, p):
                    xm = xmp.tile([128, CHUNK], BF16, tag="xm")
                    nc.vector.tensor_tensor(xm[:], a[:], xwin_of(p), ALU.mult)
                    xms[gid] = xm

                dve_tt(1, shuf_rr, 0)            # p0 f1
                dve_tt(4, rrpe[1], 1)            # p1 f1
                for j in (2, 3):                 # p1 f2 (Pool), f3 (DVE)
                    eb = ebp.tile([128, CHUNK], BF16, tag="eb")
                    nc.vector.tensor_scalar(eb[:], rrpe[1][:], j - 0.5, None, ALU.is_ge)
                    xm = xmp.tile([128, CHUNK], BF16, tag="xm")
                    if j == 2:
                        nc.gpsimd.tensor_tensor(xm[:], eb[:], xwin_of(1), ALU.mult)
                    else:
                        nc.vector.tensor_tensor(xm[:], eb[:], xwin_of(1), ALU.mult)
                    xms[3 + j] = xm
                dve_tt(7, rrpe[2], 2)            # p2 f1
                eb2 = ebp.tile([128, CHUNK], BF16, tag="eb")
                nc.vector.tensor_scalar(eb2[:], rrpe[2][:], 1.5, None, ALU.is_ge)
                dve_tt(8, eb2, 2)                # p2 f2 = E2*x
                dve_tt(9, sg[(2, 3)], 2)         # p2 f3 = s3*x
                dve_tt(11, rrpe[3], 3)           # p3 f1
                dve_tt(12, sg[(3, 2)], 3)        # p3 f2 = s2*x
                dve_tt(13, sg[(3, 3)], 3)        # p3 f3 = s3*x

                # ---- next chunk's p0 shuffle+TS (on DVE, at end) ----
                if ci + 1 < NCHUNK:
                    pipe = emit_shuf_ts(ci + 1)

                # ---- matmuls ----
                acc = psp.tile([O, CHUNK], F32)
                for oi, gid in enumerate(MM_ORDER):
                    if gid == 0:
                        rhs = tA[:, BASE + c0 : BASE + c0 + CHUNK]
                    elif gid == 10:
                        rhs = xwin_of(2)
                    elif gid == 14:
                        rhs = xwin_of(3)
                    else:
                        rhs = xms[gid][:]
                    for h in range(CHUNK // 512):
                        nc.tensor.matmul(
                            acc[:, h * 512 : (h + 1) * 512],
                            wp[:, gid * 64 : (gid + 1) * 64],
                            bass.AP(
                                rhs.tensor,
                                rhs.offset + h * 512,
                                [list(rhs.ap[0])] + [[1, 512]],
                            ),
                            start=(oi == 0),
                            stop=(oi == NGRP - 1),
                        )
                osb = outp.tile([O, CHUNK], F32, tag="osb")
                nc.scalar.activation(osb[:], acc[:], AF.Copy)
                nc.sync.dma_start(out_d[:, c0 : c0 + CHUNK], osb[:])

    noop_cls = type(nc.sync.nop().ins)
    _split_excess_waits(nc, noop_cls, max_waits=1)
    return nc


def _bf(a):
    return a.astype(ml_dtypes.bfloat16).astype(np.float32)


def _prep_weights(w0, w1, w2):
    # basis-transformed weights; see module docstring.
    # E-basis (p0,p1):  V1=W2, V2=W1-2W2, V3=W0-W1-W2
    # p2 {f1,E2,s3,xr}: T1=W2, T2=W1-2W2, T3=.5(W0-W1-W2), T4=T3
    # p3 {f1,s2,s3,xr}: U1=W2, U2=.5W1-W2, U3=.5(W0-W1-W2), U4=U2+U3
    #   (U4/T4 computed from the bf16-rounded terms so phi=0 pixels
    #    cancel exactly in the f32 PSUM accumulation)
    ws = [w0.reshape(O, C, 9), w1.reshape(O, C, 9), w2.reshape(O, C, 9)]
    W0, W1, W2 = ws
    wp = np.zeros((128, NGRP * 64), dtype=np.float32)

    def put(gi, ka, kb, arr):
        wp[0:64, gi * 64 : (gi + 1) * 64] = arr[:, :, ka].T
        if kb is not None:
            wp[64:128, gi * 64 : (gi + 1) * 64] = arr[:, :, kb].T

    put(0, 4, None, W1)  # center
    V = [W2, W1 - 2 * W2, W0 - W1 - W2]
    for p in (0, 1):
        ka, kb = PAIRS[p]
        for j in range(3):
            put(1 + p * 3 + j, ka, kb, V[j])
    ka, kb = PAIRS[2]
    T3 = _bf(0.5 * (W0 - W1 - W2))
    for gi, arr in ((7, W2), (8, W1 - 2 * W2), (9, T3), (10, T3)):
        put(gi, ka, kb, arr)
    ka, kb = PAIRS[3]
    U2 = _bf(0.5 * W1 - W2)
    U3 = _bf(0.5 * (W0 - W1 - W2))
    for gi, arr in ((11, W2), (12, U2), (13, U3), (14, U2 + U3)):
        put(gi, ka, kb, arr)
    return wp.astype(ml_dtypes.bfloat16)


def _prep_sel():
    # selection stationaries over moving rc[0:6] (rows = taps 7,8,2,5,3,6):
    # block bi covers pattern 1+bi: upper <- row 2bi, lower <- row 2bi+1
    sel = np.zeros((6, 384), dtype=np.float32)
    for bi in range(3):
        sel[2 * bi, bi * 128 : bi * 128 + 64] = 1.0
        sel[2 * bi + 1, bi * 128 + 64 : bi * 128 + 128] = 1.0
    return sel.astype(ml_dtypes.bfloat16)


def kernel(x, depth, fx, weight_0, weight_1, weight_2, _trace=False):
    x = np.asarray(x, dtype=np.float32)
    depth = np.asarray(depth, dtype=np.float32)
    fx = np.asarray(fx, dtype=np.float32)
    wp = _prep_weights(
        np.asarray(weight_0, np.float32),
        np.asarray(weight_1, np.float32),
        np.asarray(weight_2, np.float32),
    )
    sel = _prep_sel()
    in_maps = []
    for i in range(N_IMG):
        in_maps.append(
            {
                "x": np.ascontiguousarray(x[i].reshape(C, L)),
                "depth": np.ascontiguousarray(depth[i, 0]),
                "rfx": np.full((128, 1), 1.0 / fx[i], dtype=np.float32),
                "wp": wp,
                "sel": sel,
            }
        )
    nc = _build_graph()
    res = run_bass_kernel_spmd(nc, in_maps, core_ids=list(range(N_IMG)), trace=_trace)
    out = np.stack([res.results[i]["out"].reshape(O, H, W) for i in range(N_IMG)])
    if _trace:
        return out.astype(np.float32), res
    return out.astype(np.float32)


if __name__ == "__main__":
    rng = np.random.default_rng(0)
    ins = {
        "x": rng.standard_normal((N_IMG, C, H, W), dtype=np.float32),
        "depth": (1.0 + 9.0 * rng.random((N_IMG, 1, H, W))).astype(np.float32),
        "fx": (400.0 + 200.0 * rng.random(N_IMG)).astype(np.float32),
        "weight_0": rng.standard_normal((O, C, 3, 3), dtype=np.float32) * 0.04,
        "weight_1": rng.standard_normal((O, C, 3, 3), dtype=np.float32) * 0.04,
        "weight_2": rng.standard_normal((O, C, 3, 3), dtype=np.float32) * 0.04,
    }
    out = kernel(**ins)
    print("ran ok", out.shape, out.dtype)
